# revision 1
# baseline (speedup 1.0000x reference)
"""Multi-Head Latent Attention (MLA) Trainium2 kernel.

Problem (hardcoded): B=2, S=2048, D_MODEL=2048, H=16, HEAD_DIM=128,
D_LATENT=512 (D_QK=256 / D_V=256), ROPE_DIM=64, fp32 in/out.

Reference semantics: q = concat([q_no_rope(1024), q_rope(1024)]).reshape(16
heads x 128), so heads 0-7 take both 64-dim halves from the latent
decompression and heads 8-15 take both halves from the rope projection of x;
RoPE rotates dims 64:128 of every head.

Sharding: 8 cores = 2 batches x 4 head-groups; core (b, hg) owns heads
[2hg, 2hg+1, 8+2hg, 8+2hg+1] (2 decompression + 2 rope-projection heads),
computes the shared latent for its batch redundantly, and produces a partial
output projection (its heads' rows of W_out), transposed [e, q]. The host
sums the 4 partials per batch.

On-chip, everything is feature-major so matmuls contract over partitions;
matmul operands are float32r (full-rate fp32, ~tf32 accuracy). Pipeline:
  pass0:   latT = W_comp^T @ x^T           (c_qk rows 0:256, c_v 256:512)
  phase2:  qkT[dec][0:128]  = W_{q,k}_dec^T @ c_qkT   (per-head full tiles)
           v_nat[S,512]     = c_v @ W_v slice          (natural layout)
  pass1:   qkT[rope][0:128] = W_rope_{q,k}^T @ x^T
  rope:    qkT[i][64:128] = raw*cos4 + swap32(raw)*sin4s  (in place)
  attn:    per (q-chunk, head): scoresT[k,q] psum = kT^T @ qT (K=128);
           expT = exp(scoresT/sqrt(128)); ctxT += v-block^T @ expT;
           den += ones^T @ expT (all psum rows = column sums -> free
           broadcast); ctxT *= 1/den (reciprocal_approx_fast)
  out:     outT[e,q] = W_out_part^T @ ctxT, fused per q-chunk
"""

import math

import numpy as np

B = 2
S = 2048
D = 2048
H4 = 4            # heads per core
HD = 128          # head dim
DL = 512          # d_latent
DQK = 256
RD = 64           # rope dim
NC = 8            # cores

SCALE = 1.0 / math.sqrt(HD)
MM_DT_NAME = "float32r"   # matmul operand dtype tag: "float32r" or "float32"

_prog_cache = {}


def _build_program(phases=4):
    import concourse.tile as tile
    from concourse import bacc, mybir

    mm_dt = getattr(mybir.dt, MM_DT_NAME)
    f32 = mybir.dt.float32

    def bc(ap):
        return ap.bitcast(mm_dt)

    nc = bacc.Bacc("TRN2", target_bir_lowering=False, debug=False, num_devices=1)

    xT = nc.dram_tensor("xT", [D, S], mm_dt, kind="ExternalInput")
    w_big = nc.dram_tensor("w_big", [D, 1024], mm_dt, kind="ExternalInput")
    w_qk = nc.dram_tensor("w_qk", [DQK, 512], mm_dt, kind="ExternalInput")
    w_v = nc.dram_tensor("w_v", [DQK, 512], mm_dt, kind="ExternalInput")
    w_o = nc.dram_tensor("w_o", [DL, D], mm_dt, kind="ExternalInput")
    cos4_d = nc.dram_tensor("cos4", [128, S], f32, kind="ExternalInput")
    sin4s_d = nc.dram_tensor("sin4s", [64, S], f32, kind="ExternalInput")
    out_d = nc.dram_tensor("out", [D, S], f32, kind="ExternalOutput")

    NQ = S // 512    # 4 q/n chunks of 512
    NK = S // 128    # 16 k/seq chunks of 128
    KD = D // 128    # 16 contraction chunks for stage 1

    with tile.TileContext(nc, pool_alloc_mode="queue") as tc:
        import contextlib

        with contextlib.ExitStack() as ctx:
            # persistent pools (live to end of program, LIFO via ExitStack)
            ones_p = ctx.enter_context(tc.tile_pool(name="onesp", bufs=1))
            qk_p = ctx.enter_context(tc.tile_pool(name="qk", bufs=1))
            v_p = ctx.enter_context(tc.tile_pool(name="vp", bufs=1))

            ones_f32 = ones_p.tile([128, 128], f32)
            nc.gpsimd.memset(ones_f32[:], 1.0)
            ones = ones_p.tile([128, 128], mm_dt)
            nc.vector.tensor_copy(ones[:], ones_f32[:])
            warm = ones_p.tile([128, 1], f32)
            nc.scalar.activation(warm[:], ones_f32[:, 0:1],
                                 mybir.ActivationFunctionType.Exp)
            # per-head assembled q/k: rows 0:64 nr dims, 64:128 roped dims
            # qkT[0..3] = q heads 0..3, qkT[4..7] = k heads 0..3
            qkT = [qk_p.tile([128, S], mm_dt, name=f"qkT{i}", tag=f"qk{i}")
                   for i in range(8)]
            v_nat = [v_p.tile([128, 512], mm_dt, name=f"v{i}", tag=f"v{i}")
                     for i in range(NK)]

            consts_cm = tc.tile_pool(name="consts", bufs=1)
            consts = consts_cm.__enter__()
            cos4 = consts.tile([128, S], f32)
            nc.sync.dma_start(cos4[:], cos4_d.ap()[:])
            sin4s = consts.tile([64, S], f32)
            nc.sync.dma_start(sin4s[:], sin4s_d.ap()[:])

            # ---------------- phase 1: bigT = w_big^T @ xT ----------------
            # two M-half passes (weights halved to fit SBUF; xT read twice)
            # mh=0 -> latent rows (-> latT), mh=1 -> raw rope rows (-> qkT[...][64:128])
            # phase 2 is emitted between the passes so PE stays busy while
            # pass-1 weights load; one shared PSUM pool avoids pool-boundary
            # drains.
            ps12_cm = tc.tile_pool(name="ps12", bufs=8, space="PSUM")
            ps12_p = ps12_cm.__enter__()

            def stage1_pass(mh, wbig_p, xt_p):
                wbig_sb = []
                for k in range(KD):
                    w_t = wbig_p.tile([128, 512], mm_dt, name=f"wb{mh}_{k}",
                                      tag=f"wb{k}")
                    wbig_sb.append(w_t)
                for np2 in range(NQ // 2):
                    # xt loaded as [128,1024] tiles (4KB-contiguous rows —
                    # double the DMA descriptor granularity), consumed by two
                    # 512-wide n-chunks
                    xts = []
                    for k in range(KD):
                        if np2 == 0:
                            # interleaved with xt loads so k=0 work starts asap
                            nc.sync.dma_start(
                                wbig_sb[k][:],
                                w_big.ap()[k * 128:(k + 1) * 128,
                                           mh * 512:(mh + 1) * 512])
                        x_t = xt_p.tile([128, 1024], mm_dt, name="xt", tag="xt")
                        nc.sync.dma_start(
                            x_t[:], xT.ap()[k * 128:(k + 1) * 128,
                                            np2 * 1024:(np2 + 1) * 1024])
                        xts.append(x_t)
                    psums = [ps12_p.tile([128, 512], f32, name=f"ps1_{m}",
                                         tag="ps12") for m in range(8)]
                    for k in range(KD):
                        for m in range(4):
                            for sub in range(2):
                                # sub inner: consecutive matmuls share lhsT
                                nc.tensor.matmul(
                                    psums[sub * 4 + m][:],
                                    bc(wbig_sb[k][:, m * 128:(m + 1) * 128]),
                                    bc(xts[k][:, sub * 512:(sub + 1) * 512]),
                                    start=(k == 0),
                                    stop=(k == KD - 1),
                                )
                    for sub in range(2):
                        n = np2 * 2 + sub
                        nsl = slice(n * 512, (n + 1) * 512)
                        for m in range(4):
                            if mh == 0:
                                nc.vector.tensor_copy(latn[m][n][:],
                                                      psums[sub * 4 + m][:])
                            else:
                                # rope-proj heads: local q heads 2,3 / k heads 2,3
                                dst = qkT[[2, 3, 6, 7][m]]
                                nc.vector.tensor_copy(dst[:, nsl],
                                                      psums[sub * 4 + m][:])

            wbig_cm = tc.tile_pool(name="wbig", bufs=1)
            wbig_p = wbig_cm.__enter__()
            xt_cm = tc.tile_pool(name="xt", bufs=5)
            xt_p = xt_cm.__enter__()
            # phase-2 weight pool opened up front (stable ring placement);
            # the loads themselves are emitted right after pass-0 so they
            # don't delay the first stage-1 tiles
            wdec_cm = tc.tile_pool(name="wdec", bufs=1)
            wdec_p = wdec_cm.__enter__()
            # latent lives only pass0 -> phase 2; nested inside wbig/xt so the
            # rope scratch can reuse its space afterwards
            lat_cm = tc.tile_pool(name="lat", bufs=1)
            lat_p = lat_cm.__enter__()
            # per-(l, n-chunk) tiles so phase 2 can start before pass 0 ends
            latn = [[lat_p.tile([128, 512], mm_dt, name=f"latT{i}_{n}",
                                tag=f"lat{i}_{n}") for n in range(NQ)]
                    for i in range(4)]
            stage1_pass(0, wbig_p, xt_p)
            wqk_sb = []
            for l in range(2):
                w_t = wdec_p.tile([128, 512], mm_dt, name=f"wqk{l}",
                                  tag=f"wqk{l}")
                nc.sync.dma_start(w_t[:], w_qk.ap()[l * 128:(l + 1) * 128, :])
                wqk_sb.append(w_t)
            wv_sb = []
            for l in range(2):
                w_t = wdec_p.tile([128, 512], mm_dt, name=f"wv{l}",
                                  tag=f"wv{l}")
                nc.sync.dma_start(w_t[:], w_v.ap()[l * 128:(l + 1) * 128, :])
                wv_sb.append(w_t)

            if phases == 1:
                for i in range(4):
                    for n in range(NQ):
                        nc.sync.dma_start(
                            out_d.ap()[i * 128:(i + 1) * 128,
                                       n * 512:(n + 1) * 512],
                            latn[i][n][:].bitcast(f32))
                for i, t in enumerate(qkT):
                    nc.sync.dma_start(out_d.ap()[512 + i * 128:512 + (i + 1) * 128, :], t[:].bitcast(f32))

            # ---------------- phase 2: nr decompression + v ----------------
            if phases >= 2:
              if True:
                ps2_p = ps12_p

                for mt in range(4):        # dec-heads: local q heads 0,1 / k heads 0,1
                    for n in range(NQ):
                        nsl = slice(n * 512, (n + 1) * 512)
                        ps = ps2_p.tile([128, 512], f32, name="ps2", tag="ps12")
                        for l in range(2):
                            nc.tensor.matmul(
                                ps[:],
                                bc(wqk_sb[l][:, mt * 128:(mt + 1) * 128]),
                                bc(latn[l][n][:]),
                                start=(l == 0), stop=(l == 1),
                            )
                        nc.vector.tensor_copy(qkT[[0, 1, 4, 5][mt]][:, nsl], ps[:])
                for sc in range(NK):
                    ps = ps2_p.tile([128, 512], f32, name="ps2v", tag="ps12")
                    for l in range(2):
                        nc.tensor.matmul(
                            ps[:],
                            bc(latn[2 + l][sc // 4][:, (sc % 4) * 128:
                                                    (sc % 4 + 1) * 128]),
                            bc(wv_sb[l][:]),
                            start=(l == 0), stop=(l == 1),
                        )
                    nc.vector.tensor_copy(v_nat[sc][:], ps[:])
            lat_cm.__exit__(None, None, None)
            wdec_cm.__exit__(None, None, None)

            # rope dec-head tiles now (they only need phase 2), so the DVE
            # work overlaps stage-1 pass 1 on PE
            def rope_tiles(swp_p, scr_p, idxs):
                # all scratch in rows 0:64 (cos/sin pattern repeats every 32
                # rows, so base-0 slices align); only the final add writes the
                # qkT rope rows at base 64
                for i in idxs:
                    t = qkT[i]
                    sw = swp_p.tile([64, S], f32, name=f"sw{i}", tag="sw")
                    nc.sync.dma_start(sw[0:32, :], t[96:128, :].bitcast(f32))
                    nc.sync.dma_start(sw[32:64, :], t[64:96, :].bitcast(f32))
                    tmp_sin = scr_p.tile([64, S], f32, name="tsin", tag="scr")
                    nc.vector.tensor_mul(tmp_sin[0:64, :], sw[0:64, :],
                                         sin4s[0:64, :])
                    tmp_cos = scr_p.tile([64, S], f32, name="tcos", tag="scr")
                    nc.vector.tensor_mul(tmp_cos[0:64, :],
                                         t[64:128, :].bitcast(f32),
                                         cos4[64:128, :])
                    nc.vector.tensor_add(t[64:128, :], tmp_cos[0:64, :],
                                         tmp_sin[0:64, :])

            with tc.tile_pool(name="swpA", bufs=2) as swp_p, \
                 tc.tile_pool(name="scrA", bufs=2) as scr_p:
                rope_tiles(swp_p, scr_p, [0, 4, 1, 5])
            stage1_pass(1, wbig_p, xt_p)
            xt_cm.__exit__(None, None, None)
            wbig_cm.__exit__(None, None, None)
            ps12_cm.__exit__(None, None, None)
            with tc.tile_pool(name="swpB", bufs=2) as swp_p, \
                 tc.tile_pool(name="scrB", bufs=2) as scr_p:
                rope_tiles(swp_p, scr_p, [2, 6, 3, 7])
            consts_cm.__exit__(None, None, None)
            if phases == 2:
                for i, t in enumerate(qkT):
                    nc.sync.dma_start(out_d.ap()[i * 128:(i + 1) * 128, :], t[:].bitcast(f32))

            if phases == 3:
                for i, t in enumerate(qkT):
                    nc.sync.dma_start(out_d.ap()[i * 128:(i + 1) * 128, :], t[:].bitcast(f32))
                for sc in range(NK):
                    nc.sync.dma_start(
                        out_d.ap()[1024 + (sc // 4) * 128:1024 + (sc // 4 + 1) * 128,
                                   (sc % 4) * 512:(sc % 4 + 1) * 512], v_nat[sc][:].bitcast(f32))

            # ---------------- phase 3+4: attention + output projection ----
            if phases >= 4:
              with tc.tile_pool(name="wo", bufs=1) as wo_p, \
                 tc.tile_pool(name="exp", bufs=4) as exp_p, \
                 tc.tile_pool(name="ctx", bufs=8) as ctx_p, \
                 tc.tile_pool(name="rden", bufs=2) as rden_p, \
                 tc.tile_pool(name="stage", bufs=4) as stage_p, \
                 tc.tile_pool(name="ps_s", bufs=2, space="PSUM") as ps_s_p, \
                 tc.tile_pool(name="ps_c", bufs=2, space="PSUM") as ps_c_p, \
                 tc.tile_pool(name="ps_d", bufs=2, space="PSUM") as ps_d_p, \
                 tc.tile_pool(name="ps_o", bufs=2, space="PSUM") as ps_o_p:
                wo_sb = []
                for kk in range(4):
                    w_t = wo_p.tile([128, D], mm_dt, name=f"wo{kk}", tag=f"wo{kk}")
                    nc.sync.dma_start(w_t[:], w_o.ap()[kk * 128:(kk + 1) * 128, :])
                    wo_sb.append(w_t)

                for qc in range(NQ):
                    qsl = slice(qc * 512, (qc + 1) * 512)
                    ctx_sb = []
                    for h in range(H4):
                        ps_ctx = ps_c_p.tile([128, 512], f32, name="psc", tag="psc")
                        ps_den = ps_d_p.tile([128, 512], f32, name="psd", tag="psd")
                        exps = []
                        # software-pipelined: scores(kc+1) is issued before
                        # av/den(kc) so PE isn't FIFO-blocked on exp(kc)
                        def emit_scores(kc):
                            ps_s = ps_s_p.tile([128, 512], f32, name="pss", tag="pss")
                            nc.tensor.matmul(
                                ps_s[:],
                                bc(qkT[4 + h][:, kc * 128:(kc + 1) * 128]),
                                bc(qkT[h][:, qsl]),
                                start=True, stop=True,
                            )
                            expT = exp_p.tile([128, 512], mm_dt, name="expT", tag="exp")
                            nc.scalar.activation(
                                expT[:], ps_s[:],
                                mybir.ActivationFunctionType.Exp, scale=SCALE)
                            exps.append(expT)

                        def emit_av(kc):
                            expT = exps[kc]
                            nc.tensor.matmul(
                                ps_ctx[:],
                                bc(v_nat[kc][:, h * 128:(h + 1) * 128]),
                                bc(expT[:]),
                                start=(kc == 0), stop=(kc == NK - 1),
                            )
                            nc.tensor.matmul(
                                ps_den[:],
                                bc(ones[:]),
                                bc(expT[:]),
                                start=(kc == 0), stop=(kc == NK - 1),
                            )

                        emit_scores(0)
                        for kc in range(1, NK):
                            emit_scores(kc)
                            emit_av(kc - 1)
                        emit_av(NK - 1)
                        rden = rden_p.tile([128, 512], f32, name="rden", tag="rden")
                        nc.vector.reciprocal_approx_fast(rden[:], ps_den[:])
                        c_t = ctx_p.tile([128, 512], mm_dt, name="ctxt", tag="ctx")
                        nc.vector.tensor_mul(c_t[:], ps_ctx[:], rden[:])
                        ctx_sb.append(c_t)
                        if phases == 5:
                            r0 = (qc * 4 + h) * 128
                            nc.sync.dma_start(out_d.ap()[r0:r0 + 128, 0:512], c_t[:].bitcast(f32))
                            nc.sync.dma_start(out_d.ap()[r0:r0 + 128, 512:1024], rden[:])
                            if qc == 0 and h == 0:
                                nc.sync.dma_start(out_d.ap()[0:128, 1024:1152], ones[:].bitcast(f32))

                    for m in range(D // 128 if phases >= 6 or phases == 4 else 0):
                        ps_o = ps_o_p.tile([128, 512], f32, name="pso", tag="pso")
                        for kk in range(4):
                            nc.tensor.matmul(
                                ps_o[:],
                                bc(wo_sb[kk][:, m * 128:(m + 1) * 128]),
                                bc(ctx_sb[kk][:]),
                                start=(kk == 0), stop=(kk == 3),
                            )
                        st = stage_p.tile([128, 512], f32, name="stg", tag="stage")
                        nc.vector.tensor_copy(st[:], ps_o[:])
                        nc.sync.dma_start(
                            out_d.ap()[m * 128:(m + 1) * 128, qsl], st[:])

    nc.compile()
    return nc


def _get_program():
    if "nc" not in _prog_cache:
        _prog_cache["nc"] = _build_program()
    return _prog_cache["nc"]


def _host_shards(x, W_comp, W_q_dec, W_k_dec, W_v_dec, W_rope_q, W_rope_k, W_out):
    half = RD // 2
    inv = 1.0 / (10000.0 ** (np.arange(0, RD, 2, dtype=np.float32) / RD))
    ang = np.arange(S, dtype=np.float32)[:, None] * inv[None, :]     # [S, 32]
    cosT = np.cos(ang).T.astype(np.float32)                          # [32, S]
    sinT = np.sin(ang).T.astype(np.float32)
    cos4 = np.ascontiguousarray(np.tile(cosT, (4, 1)))               # [128, S]
    sin4s = np.ascontiguousarray(
        np.concatenate([-sinT, sinT], axis=0))                       # [64, S]

    in_maps = []
    for c in range(NC):
        b, hg = divmod(c, 4)
        xTb = np.ascontiguousarray(x[b].T)
        w_big = np.ascontiguousarray(np.concatenate(
            [W_comp,
             W_rope_q[:, hg * 256:(hg + 1) * 256],
             W_rope_k[:, hg * 256:(hg + 1) * 256]], axis=1))
        w_qk = np.ascontiguousarray(np.concatenate(
            [W_q_dec[:, hg * 256:(hg + 1) * 256],
             W_k_dec[:, hg * 256:(hg + 1) * 256]], axis=1))
        w_v = np.ascontiguousarray(np.concatenate(
            [W_v_dec[:, hg * 256:(hg + 1) * 256],
             W_v_dec[:, 1024 + hg * 256:1024 + (hg + 1) * 256]], axis=1))
        w_o = np.ascontiguousarray(np.concatenate(
            [W_out[hg * 256:(hg + 1) * 256, :],
             W_out[1024 + hg * 256:1024 + (hg + 1) * 256, :]], axis=0))
        in_maps.append({
            "xT": xTb, "w_big": w_big, "w_qk": w_qk, "w_v": w_v, "w_o": w_o,
            "cos4": cos4, "sin4s": sin4s,
        })
    return in_maps


def kernel(x, W_comp, W_q_dec, W_k_dec, W_v_dec, W_rope_q, W_rope_k, W_out,
           _trace=False):
    from concourse import bass_utils

    x = np.asarray(x, np.float32)
    args = [np.asarray(a, np.float32)
            for a in (W_comp, W_q_dec, W_k_dec, W_v_dec,
                      W_rope_q, W_rope_k, W_out)]
    in_maps = _host_shards(x, *args)
    nc = _get_program()
    res = bass_utils.run_bass_kernel_spmd(
        nc, in_maps, core_ids=list(range(NC)), trace=_trace)
    out = np.zeros((B, S, D), np.float32)
    for c in range(NC):
        b = c // 4
        out[b] += res.results[c]["out"].T
    if _trace:
        kernel.last_exec_ns = res.exec_time_ns
    return out



# revision 6
# speedup vs baseline: 1.2383x; 1.2383x over previous
"""Multi-Head Latent Attention (MLA) Trainium2 kernel, v2 (fp16).

Problem (hardcoded): B=2, S=2048, D_MODEL=2048, H=16, HEAD_DIM=128,
D_LATENT=512 (D_QK=256 / D_V=256), ROPE_DIM=64, fp32 in/out.

Reference semantics: q = concat([q_no_rope(1024), q_rope(1024)]).reshape(16
heads x 128), so heads 0-7 take both 64-dim halves from the latent
decompression and heads 8-15 take both halves from the rope projection of x;
RoPE rotates dims 64:128 of every head.

Sharding: 8 cores = 2 batches x 4 head-groups; core (b, hg) owns heads
[2hg, 2hg+1, 8+2hg, 8+2hg+1] (2 decompression + 2 rope-projection heads),
computes the shared latent for its batch redundantly, and produces a partial
output projection (its heads' rows of W_out), transposed [e, q]. The host
sums the 4 partials per batch.

v2 changes vs the fp32r baseline:
  - all matmul operands fp16 (host converts; halves HBM traffic, same PE
    rate, validated rel err ~1e-3)
  - stage1 single pass, m-block outer with all of xT resident in SBUF
    (xT read once instead of twice)
  - softmax denominator accumulated on DVE (fp16 2x) + one small
    ones-matmul per (qc,h) instead of 256 full ones-matmuls on PE
  - exp over [128,1024] kc-pair tiles (half the activation count)
  - exp uses bias=-3 shift (cancels in softmax; keeps fp16 range safe)
  - psum tiles span banks ([128,2048] stage1, [128,1024] attn) to cut
    instruction/semaphore counts
"""

import math

import numpy as np

B = 2
S = 2048
D = 2048
H4 = 4            # heads per core
HD = 128          # head dim
DL = 512          # d_latent
DQK = 256
RD = 64           # rope dim
NC = 8            # cores

NQ = S // 512     # 4 q chunks of 512
NK = S // 128     # 16 k chunks of 128
NP = NK // 2      # 8 kc pairs
KD = D // 128     # 16 contraction chunks for stage 1

SCALE = 1.0 / math.sqrt(HD)
EXP_SHIFT = -3.0  # exp(s*SCALE - 3): cancels in softmax, keeps fp16 range

_prog_cache = {}


def _build_program():
    import contextlib

    import concourse.tile as tile
    from concourse import bacc, mybir

    f16 = mybir.dt.float16
    f32 = mybir.dt.float32
    EXP = mybir.ActivationFunctionType.Exp

    nc = bacc.Bacc("TRN2", target_bir_lowering=False, debug=False,
                   num_devices=1)

    xT = nc.dram_tensor("xT", [D, S], f16, kind="ExternalInput")
    w_big = nc.dram_tensor("w_big", [D, 1024], f16, kind="ExternalInput")
    w_qk = nc.dram_tensor("w_qk", [DQK, 512], f16, kind="ExternalInput")
    w_v = nc.dram_tensor("w_v", [DQK, 512], f16, kind="ExternalInput")
    w_o = nc.dram_tensor("w_o", [DL, D], f16, kind="ExternalInput")
    cos4_d = nc.dram_tensor("cos4", [128, S], f16, kind="ExternalInput")
    sin4s_d = nc.dram_tensor("sin4s", [64, S], f16, kind="ExternalInput")
    out_d = nc.dram_tensor("out", [D, S], f16, kind="ExternalOutput")

    with tile.TileContext(nc, pool_alloc_mode="queue") as tc:
        with contextlib.ExitStack() as ctx:
            # ---------------- persistent pools ----------------
            ones_p = ctx.enter_context(tc.tile_pool(name="onesp", bufs=1))
            qk_p = ctx.enter_context(tc.tile_pool(name="qk", bufs=1))
            v_p = ctx.enter_context(tc.tile_pool(name="vp", bufs=1))
            consts_p = ctx.enter_context(tc.tile_pool(name="consts", bufs=1))
            wdec_p = ctx.enter_context(tc.tile_pool(name="wdec", bufs=1))

            ones_f32 = ones_p.tile([128, 128], f32)
            nc.gpsimd.memset(ones_f32[:], 1.0)
            ones16 = ones_p.tile([128, 128], f16)
            nc.vector.tensor_copy(ones16[:], ones_f32[:])
            warm = ones_p.tile([128, 1], f32)
            nc.scalar.activation(warm[:], ones_f32[:, 0:1], EXP)
            shift = ones_p.tile([128, 1], f32)
            nc.gpsimd.memset(shift[:], EXP_SHIFT)

            # qkT[0..3] = q heads 0..3, qkT[4..7] = k heads 0..3
            # (local heads 0,1 = dec heads; 2,3 = rope-proj heads)
            qkT = [qk_p.tile([128, S], f16, name=f"qkT{i}", tag=f"qk{i}")
                   for i in range(8)]
            # v pair tiles: v_pair[p] = v_nat[2p] | v_nat[2p+1]
            v_pair = [v_p.tile([128, 1024], f16, name=f"v{p}", tag=f"v{p}")
                      for p in range(NP)]

            cos4 = consts_p.tile([128, S], f16)
            nc.sync.dma_start(cos4[:], cos4_d.ap()[:])
            sin4s = consts_p.tile([64, S], f16)
            nc.sync.dma_start(sin4s[:], sin4s_d.ap()[:])

            wqk_sb = []
            for l in range(2):
                w_t = wdec_p.tile([128, 512], f16, name=f"wqk{l}")
                nc.sync.dma_start(w_t[:], w_qk.ap()[l * 128:(l + 1) * 128, :])
                wqk_sb.append(w_t)
            wv_sb = []
            for l in range(2):
                w_t = wdec_p.tile([128, 512], f16, name=f"wv{l}")
                nc.sync.dma_start(w_t[:], w_v.ap()[l * 128:(l + 1) * 128, :])
                wv_sb.append(w_t)

            # ---------------- phase A: stage1 + dec + v + rope ----------
            # latn[i] = latent rows [128i,128i+128) x [S], fp16
            # (i=0,1: c_qk; i=2,3: c_v)
            lat_cm = tc.tile_pool(name="lat", bufs=1)
            lat_p = lat_cm.__enter__()
            latn = [lat_p.tile([128, S], f16, name=f"latT{i}", tag=f"lat{i}")
                    for i in range(4)]

            xt_cm = tc.tile_pool(name="xt", bufs=1)
            xt_p = xt_cm.__enter__()
            wbig_cm = tc.tile_pool(name="wbig", bufs=1)
            wbig_p = wbig_cm.__enter__()
            wbig_sb = []
            xt_sb = []
            for k in range(KD):
                w_t = wbig_p.tile([128, 1024], f16, name=f"wb{k}")
                nc.sync.dma_start(
                    w_t[:], w_big.ap()[k * 128:(k + 1) * 128, :])
                wbig_sb.append(w_t)
                x_t = xt_p.tile([128, S], f16, name=f"xt{k}")
                nc.sync.dma_start(x_t[:], xT.ap()[k * 128:(k + 1) * 128, :])
                xt_sb.append(x_t)

            psA_cm = tc.tile_pool(name="psA", bufs=2, space="PSUM")
            psA_p = psA_cm.__enter__()

            sw_cm = tc.tile_pool(name="swp", bufs=2)
            sw_p = sw_cm.__enter__()
            scr_cm = tc.tile_pool(name="scr", bufs=2)
            scr_p = scr_cm.__enter__()

            def rope_tile(i):
                # in-place RoPE on rows 64:128 of qkT[i]
                t = qkT[i]
                sw = sw_p.tile([64, S], f16, name=f"sw{i}", tag="sw")
                nc.sync.dma_start(sw[0:32, :], t[96:128, :])
                nc.sync.dma_start(sw[32:64, :], t[64:96, :])
                tmp_sin = scr_p.tile([64, S], f16, name="tsin", tag="scr")
                nc.vector.tensor_mul(tmp_sin[:], sw[:], sin4s[:])
                tmp_cos = scr_p.tile([64, S], f16, name="tcos", tag="scr")
                nc.vector.tensor_mul(tmp_cos[:], t[64:128, :], cos4[64:128, :])
                nc.vector.tensor_add(t[64:128, :], tmp_cos[:], tmp_sin[:])

            def stage1_m(m, dst_cast):
                # one m-block: psum [128,2048] accumulated over all KD chunks
                ps = psA_p.tile([128, 2048], f32, name=f"ps_m{m}", tag="psA")
                for k in range(KD):
                    for sub in range(4):
                        nc.tensor.matmul(
                            ps[:, sub * 512:(sub + 1) * 512],
                            wbig_sb[k][:, m * 128:(m + 1) * 128],
                            xt_sb[k][:, sub * 512:(sub + 1) * 512],
                            start=(k == 0), stop=(k == KD - 1),
                        )
                dst_cast(ps)

            def dec_mt(mt):
                # q/k decompression for dec head mt -> qkT[[0,1,4,5][mt]]
                ps = psA_p.tile([128, 2048], f32, name=f"ps_d{mt}", tag="psA")
                for l in range(2):
                    for n in range(NQ):
                        nc.tensor.matmul(
                            ps[:, n * 512:(n + 1) * 512],
                            wqk_sb[l][:, mt * 128:(mt + 1) * 128],
                            latn[l][:, n * 512:(n + 1) * 512],
                            start=(l == 0), stop=(l == 1),
                        )
                dst = qkT[[0, 1, 4, 5][mt]]
                nc.vector.tensor_copy(dst[:], ps[:])

            # emission: m0, m1 (c_qk) -> dec -> m2, m3 (c_v) -> v -> m4..m7
            for m in (0, 1):
                stage1_m(m, lambda ps, i=m: nc.vector.tensor_copy(
                    latn[i][:], ps[:]))
            for mt in range(4):
                dec_mt(mt)
                rope_tile([0, 1, 4, 5][mt])
            for m in (2, 3):
                stage1_m(m, lambda ps, i=m: nc.vector.tensor_copy(
                    latn[i][:], ps[:]))
            # v decompression: 4 sc per psum tile
            for scg in range(4):
                ps = psA_p.tile([128, 2048], f32, name=f"ps_v{scg}", tag="psA")
                for j in range(4):
                    sc = scg * 4 + j
                    for l in range(2):
                        nc.tensor.matmul(
                            ps[:, j * 512:(j + 1) * 512],
                            latn[2 + l][:, sc * 128:(sc + 1) * 128],
                            wv_sb[l][:],
                            start=(l == 0), stop=(l == 1),
                        )
                for j2 in range(2):
                    sc0 = scg * 4 + j2 * 2
                    nc.vector.tensor_copy(
                        v_pair[sc0 // 2][:],
                        ps[:, j2 * 1024:(j2 + 1) * 1024])
            for m in (4, 5, 6, 7):
                dst = qkT[[2, 3, 6, 7][m - 4]]
                stage1_m(m, lambda ps, t=dst: nc.vector.tensor_copy(
                    t[:], ps[:]))
                rope_tile([2, 3, 6, 7][m - 4])

            scr_cm.__exit__(None, None, None)
            sw_cm.__exit__(None, None, None)
            psA_cm.__exit__(None, None, None)
            wbig_cm.__exit__(None, None, None)
            xt_cm.__exit__(None, None, None)
            lat_cm.__exit__(None, None, None)

            # ---------------- phase B: attention + out projection --------
            with tc.tile_pool(name="wo", bufs=1) as wo_p, \
                 tc.tile_pool(name="exp", bufs=3) as exp_p, \
                 tc.tile_pool(name="den", bufs=2) as den_p, \
                 tc.tile_pool(name="rden", bufs=2) as rden_p, \
                 tc.tile_pool(name="ctx", bufs=8) as ctx_p, \
                 tc.tile_pool(name="stage", bufs=3) as stage_p, \
                 tc.tile_pool(name="ps_s", bufs=2, space="PSUM") as ps_s_p, \
                 tc.tile_pool(name="ps_c", bufs=2, space="PSUM") as ps_c_p, \
                 tc.tile_pool(name="ps_d", bufs=2, space="PSUM") as ps_d_p:
                wo_sb = []
                for kk in range(4):
                    w_t = wo_p.tile([128, D], f16, name=f"wo{kk}")
                    nc.sync.dma_start(
                        w_t[:], w_o.ap()[kk * 128:(kk + 1) * 128, :])
                    wo_sb.append(w_t)

                for qc in range(NQ):
                    qsl = slice(qc * 512, (qc + 1) * 512)
                    ctx_sb = []
                    for h in range(H4):
                        ps_ctx = ps_c_p.tile([128, 512], f32, name="psc",
                                             tag="psc")
                        acc = den_p.tile([128, 1024], f16, name="acc",
                                         tag="acc")
                        exps = []

                        def emit_scores(p):
                            ps = ps_s_p.tile([128, 1024], f32, name="pss",
                                             tag="pss")
                            for j in range(2):
                                kc = 2 * p + j
                                nc.tensor.matmul(
                                    ps[:, j * 512:(j + 1) * 512],
                                    qkT[4 + h][:, kc * 128:(kc + 1) * 128],
                                    qkT[h][:, qsl],
                                    start=True, stop=True,
                                )
                            e = exp_p.tile([128, 1024], f16, name="expT",
                                           tag="exp")
                            nc.scalar.activation(e[:], ps[:], EXP,
                                                 bias=shift[:], scale=SCALE)
                            exps.append(e)

                        def emit_av(p):
                            e = exps[p]
                            for j in range(2):
                                kc = 2 * p + j
                                nc.tensor.matmul(
                                    ps_ctx[:],
                                    v_pair[p][:, j * 512 + h * 128:
                                              j * 512 + (h + 1) * 128],
                                    e[:, j * 512:(j + 1) * 512],
                                    start=(kc == 0), stop=(kc == NK - 1),
                                )
                            if p == 0:
                                nc.vector.tensor_copy(acc[:], e[:])
                            else:
                                nc.vector.tensor_add(acc[:], acc[:], e[:])

                        emit_scores(0)
                        for p in range(1, NP):
                            emit_scores(p)
                            emit_av(p - 1)
                        emit_av(NP - 1)

                        fold = den_p.tile([128, 512], f16, name="fold",
                                          tag="acc")
                        nc.vector.tensor_add(fold[:], acc[:, 0:512],
                                             acc[:, 512:1024])
                        ps_den = ps_d_p.tile([128, 512], f32, name="psd",
                                             tag="psd")
                        nc.tensor.matmul(ps_den[:], ones16[:], fold[:],
                                         start=True, stop=True)
                        rden = rden_p.tile([128, 512], f32, name="rden",
                                           tag="rden")
                        nc.vector.reciprocal_approx_fast(rden[:], ps_den[:])
                        c_t = ctx_p.tile([128, 512], f16, name="ctxt",
                                         tag="ctx")
                        nc.vector.tensor_mul(c_t[:], ps_ctx[:], rden[:])
                        ctx_sb.append(c_t)

                    # output projection for this q chunk
                    for mp in range(8):
                        ps = ps_s_p.tile([128, 1024], f32, name="pso",
                                         tag="pss")
                        for half in range(2):
                            m = 2 * mp + half
                            for kk in range(4):
                                nc.tensor.matmul(
                                    ps[:, half * 512:(half + 1) * 512],
                                    wo_sb[kk][:, m * 128:(m + 1) * 128],
                                    ctx_sb[kk][:],
                                    start=(kk == 0), stop=(kk == 3),
                                )
                        st = stage_p.tile([128, 1024], f16, name="stg",
                                          tag="stage")
                        nc.vector.tensor_copy(st[:], ps[:])
                        for half in range(2):
                            m = 2 * mp + half
                            nc.sync.dma_start(
                                out_d.ap()[m * 128:(m + 1) * 128, qsl],
                                st[:, half * 512:(half + 1) * 512])

    nc.compile()
    return nc


def _get_program():
    if "nc" not in _prog_cache:
        _prog_cache["nc"] = _build_program()
    return _prog_cache["nc"]


def _host_shards(x, W_comp, W_q_dec, W_k_dec, W_v_dec, W_rope_q, W_rope_k,
                 W_out):
    inv = 1.0 / (10000.0 ** (np.arange(0, RD, 2, dtype=np.float32) / RD))
    ang = np.arange(S, dtype=np.float32)[:, None] * inv[None, :]  # [S, 32]
    cosT = np.cos(ang).T.astype(np.float32)                       # [32, S]
    sinT = np.sin(ang).T.astype(np.float32)
    cos4 = np.ascontiguousarray(np.tile(cosT, (4, 1))).astype(np.float16)
    sin4s = np.ascontiguousarray(
        np.concatenate([-sinT, sinT], axis=0)).astype(np.float16)

    in_maps = []
    for c in range(NC):
        b, hg = divmod(c, 4)
        xTb = np.ascontiguousarray(x[b].T.astype(np.float16))
        w_big = np.ascontiguousarray(np.concatenate(
            [W_comp,
             W_rope_q[:, hg * 256:(hg + 1) * 256],
             W_rope_k[:, hg * 256:(hg + 1) * 256]],
            axis=1).astype(np.float16))
        w_qk = np.ascontiguousarray(np.concatenate(
            [W_q_dec[:, hg * 256:(hg + 1) * 256],
             W_k_dec[:, hg * 256:(hg + 1) * 256]],
            axis=1).astype(np.float16))
        w_v = np.ascontiguousarray(np.concatenate(
            [W_v_dec[:, hg * 256:(hg + 1) * 256],
             W_v_dec[:, 1024 + hg * 256:1024 + (hg + 1) * 256]],
            axis=1).astype(np.float16))
        w_o = np.ascontiguousarray(np.concatenate(
            [W_out[hg * 256:(hg + 1) * 256, :],
             W_out[1024 + hg * 256:1024 + (hg + 1) * 256, :]],
            axis=0).astype(np.float16))
        in_maps.append({
            "xT": xTb, "w_big": w_big, "w_qk": w_qk, "w_v": w_v, "w_o": w_o,
            "cos4": cos4, "sin4s": sin4s,
        })
    return in_maps


def kernel(x, W_comp, W_q_dec, W_k_dec, W_v_dec, W_rope_q, W_rope_k, W_out,
           _trace=False):
    from concourse import bass_utils

    x = np.asarray(x, np.float32)
    args = [np.asarray(a, np.float32)
            for a in (W_comp, W_q_dec, W_k_dec, W_v_dec,
                      W_rope_q, W_rope_k, W_out)]
    in_maps = _host_shards(x, *args)
    nc = _get_program()
    res = bass_utils.run_bass_kernel_spmd(
        nc, in_maps, core_ids=list(range(NC)), trace=_trace)
    out = np.zeros((B, S, D), np.float32)
    for c in range(NC):
        b = c // 4
        out[b] += res.results[c]["out"].T.astype(np.float32)
    if _trace:
        kernel.last_exec_ns = res.exec_time_ns
    return out
